# revision 1
# baseline (speedup 1.0000x reference)
"""BiLSTM tagger (B=32, S=256, E=H=512, V=50000, T=64) on 8 Trainium2 cores.

Strategy (single SPMD launch):
  - Cores 0-3 run the forward direction, cores 4-7 the backward direction
    (clones within each group). The program is identical on all cores;
    direction comes only from the per-core input data.
  - The embedding gather happens on host (pure data movement); the device
    gets xs^T pre-packed in bf16.
  - Per layer: the x-projection (gates = Wx.T @ x, bias folded in via an
    augmented ones-row) runs as a large bf16 GEMM (matmul_tile_kernel);
    the sequential LSTM recurrence runs as a 256-step orientation-B scan
    (weights stationary per step, batch streamed, N=32 matmuls).
  - The two directions exchange h-sequences with pairwise AllGathers.
    Time-reversal of the peer sequence is never materialized: the next
    projection is split into an "own" GEMM (time-aligned) and a "peer" GEMM
    over the raw AllGather buffer whose non-peer rows are zeroed in the
    host-packed weights; the scan consumes that second stream with reversed
    block/step indexing.
  - Tag projection: same own/peer split; the small [64, S*B] peer result is
    reversed with a negative-stride DMA and added on-device.

Numerics: matmuls in bf16 with fp32 PSUM accumulation; gate activations,
cell state and elementwise updates in fp32; h stored bf16.
"""

import numpy as np
import ml_dtypes
import concourse.bass as bass
import concourse.bacc as bacc
import concourse.mybir as mybir
from concourse.tile import TileContext
from concourse.kernels.tile_matmul import matmul_tile_kernel
from concourse.bass_utils import run_bass_kernel_spmd

F32 = mybir.dt.float32
BF16 = mybir.dt.bfloat16
AF = mybir.ActivationFunctionType

S, B, E, H, V, T = 256, 32, 512, 512, 50000, 64
BLK = 8
REPLICA_GROUPS = [[0, 4], [1, 5], [2, 6], [3, 7]]
GATE_PERM = [0, 1, 3, 2]  # reference gate order [f,i,c,o] -> ours [f,i,o,ch]

ts = lambda i, n: slice(i * n, (i + 1) * n)


def _scan_cell(nc, tc, pools, wh_in, gx_ap, hout_ap, gxb_ap=None):
    """LSTM scan, orientation-B: psum bank [128,512] holds the 4 gate blocks
    [f|i|o|ch] in "chunked" layout (elem (p, 32a+b) = gate-dim 128a+p, batch b).
    gx_ap [2048, S*32] f32 is the precomputed x-projection (+bias), consumed
    time-aligned; gxb_ap is an optional second stream consumed time-reversed."""
    wpool, gxpool, state, gbuf, hring, psum = pools
    nb = S // BLK
    wt = []
    for k in range(4):
        w = wpool.tile([128, 2048], BF16, tag=f"wt{k}")
        nc.sync.dma_start(out=w[:, :], in_=wh_in[128 * k:128 * (k + 1), :])
        wt.append(w)
    h = state.tile([128, 128], BF16, tag="hst")
    c = state.tile([128, 128], F32, tag="cst")
    nc.vector.memset(h[:, :], 0.0)
    nc.vector.memset(c[:, :], 0.0)
    gsrc = gx_ap.rearrange("(m p) (s b) -> m p s b", p=128, b=32)
    gsrcb = gxb_ap.rearrange("(m p) (s b) -> m p s b", p=128, b=32) if gxb_ap is not None else None
    hdst = hout_ap.rearrange("(k p) (s b) -> k p s b", p=128, b=32)
    for blk in range(nb):
        gxt = gxpool.tile([128, BLK * 512], F32, tag="gx")
        dst = gxt[:, :].rearrange("p (u m b) -> m p u b", m=16, b=32)
        for m in range(16):
            nc.sync.dma_start(out=dst[m, :, :, :], in_=gsrc[m, :, ts(blk, BLK), :])
        if gsrcb is not None:
            gxtb = gxpool.tile([128, BLK * 512], F32, tag="gxb")
            dstb = gxtb[:, :].rearrange("p (u m b) -> m p u b", m=16, b=32)
            for m in range(16):
                nc.sync.dma_start(out=dstb[m, :, :, :], in_=gsrcb[m, :, ts(nb - 1 - blk, BLK), :])
        hb = hring.tile([128, BLK * 128], BF16, tag="hb")
        for u in range(BLK):
            ps = psum.tile([128, 512], F32, tag="ps")
            gxs = gxt[:, u * 512:(u + 1) * 512]
            for m in range(16):
                for k in range(4):
                    nc.tensor.matmul(
                        ps[:, 32 * m:32 * m + 32],
                        lhsT=wt[k][:, 128 * m:128 * (m + 1)],
                        rhs=h[:, 32 * k:32 * k + 32],
                        start=(k == 0), stop=(k == 3),
                    )
            g = gbuf.tile([128, 512], F32, tag="g")
            for gi in range(4):
                sl = slice(128 * gi, 128 * (gi + 1))
                nc.vector.tensor_add(out=g[:, sl], in0=ps[:, sl], in1=gxs[:, sl])
            if gsrcb is not None:
                gxsb = gxtb[:, (BLK - 1 - u) * 512:(BLK - u) * 512]
                for gi in range(4):
                    sl = slice(128 * gi, 128 * (gi + 1))
                    nc.vector.tensor_add(out=g[:, sl], in0=g[:, sl], in1=gxsb[:, sl])
            act = gbuf.tile([128, 512], F32, tag="act")
            nc.scalar.activation(act[:, 0:384], g[:, 0:384], AF.Sigmoid)
            nc.scalar.activation(act[:, 384:512], g[:, 384:512], AF.Tanh)
            t1 = gbuf.tile([128, 128], F32, tag="t1")
            t2 = gbuf.tile([128, 128], F32, tag="t2")
            nc.vector.tensor_mul(out=t1[:, :], in0=act[:, 0:128], in1=c[:, :])
            nc.vector.tensor_mul(out=t2[:, :], in0=act[:, 128:256], in1=act[:, 384:512])
            nc.vector.tensor_add(out=c[:, :], in0=t1[:, :], in1=t2[:, :])
            nc.scalar.activation(t1[:, :], c[:, :], AF.Tanh)
            nc.vector.tensor_mul(out=h[:, :], in0=act[:, 256:384], in1=t1[:, :])
            nc.vector.tensor_copy(out=hb[:, u * 128:(u + 1) * 128], in_=h[:, :])
        hsrc = hb[:, :].rearrange("p (u k b) -> k p u b", k=4, b=32)
        for k in range(4):
            nc.sync.dma_start(out=hdst[k, :, ts(blk, BLK), :], in_=hsrc[k, :, :, :])


def _build_full():
    N = S * 32
    nc = bacc.Bacc("TRN2", target_bir_lowering=False, debug=False, num_devices=8)
    xsT = nc.dram_tensor("xsT", [640, N], BF16, kind="ExternalInput")
    wx1 = nc.dram_tensor("wx1", [640, 2048], BF16, kind="ExternalInput")
    wh1 = nc.dram_tensor("wh1", [512, 2048], BF16, kind="ExternalInput")
    wx2a = nc.dram_tensor("wx2a", [640, 2048], BF16, kind="ExternalInput")
    wx2b = nc.dram_tensor("wx2b", [1024, 2048], BF16, kind="ExternalInput")
    wh2 = nc.dram_tensor("wh2", [512, 2048], BF16, kind="ExternalInput")
    wtaga = nc.dram_tensor("wtaga", [512, 64], BF16, kind="ExternalInput")
    wtagb = nc.dram_tensor("wtagb", [1024, 64], BF16, kind="ExternalInput")
    tags = nc.dram_tensor("tags", [64, N], F32, kind="ExternalOutput")

    with TileContext(nc) as tc:
        with (
            tc.tile_pool(name="dram", bufs=1, space="DRAM") as dram,
            tc.tile_pool(name="cons", bufs=1) as cons,
            tc.tile_pool(name="wpool", bufs=1) as wpool,
            tc.tile_pool(name="gxpool", bufs=2) as gxpool,
            tc.tile_pool(name="state", bufs=1) as state,
            tc.tile_pool(name="gbuf", bufs=2) as gbuf,
            tc.tile_pool(name="hring", bufs=2) as hring,
            tc.tile_pool(name="psum", bufs=2, space="PSUM") as psum,
        ):
            pools = (wpool, gxpool, state, gbuf, hring, psum)

            gx1 = dram.tile([2048, N], F32, tag="gx1")
            matmul_tile_kernel(tc, wx1[:, :], xsT[:, :], gx1[:, :])
            hout1 = dram.tile([512, N], BF16, tag="hout1")
            _scan_cell(nc, tc, pools, wh1, gx1[:, :], hout1[:, :])

            ag1 = dram.tile([1024, N], BF16, tag="ag1")
            nc.gpsimd.collective_compute(
                "AllGather", mybir.AluOpType.bypass,
                replica_groups=REPLICA_GROUPS,
                ins=[hout1.opt()], outs=[ag1.opt()],
            )
            catA = dram.tile([640, N], BF16, tag="catA")
            nc.sync.dma_start(out=catA[0:512, :], in_=hout1[:, :])
            ones = cons.tile([128, N], BF16, tag="aug")
            nc.vector.memset(ones[:, :], 0.0)
            nc.vector.memset(ones[0:1, :], 1.0)
            nc.sync.dma_start(out=catA[512:640, :], in_=ones[:, :])

            gx2a = dram.tile([2048, N], F32, tag="gx2a")
            matmul_tile_kernel(tc, wx2a[:, :], catA[:, :], gx2a[:, :])
            gx2b = dram.tile([2048, N], F32, tag="gx2b")
            matmul_tile_kernel(tc, wx2b[:, :], ag1[:, :], gx2b[:, :])

            hout2 = dram.tile([512, N], BF16, tag="hout2")
            _scan_cell(nc, tc, pools, wh2, gx2a[:, :], hout2[:, :], gxb_ap=gx2b[:, :])

            ag2 = dram.tile([1024, N], BF16, tag="ag2")
            nc.gpsimd.collective_compute(
                "AllGather", mybir.AluOpType.bypass,
                replica_groups=REPLICA_GROUPS,
                ins=[hout2.opt()], outs=[ag2.opt()],
            )
            tagA = dram.tile([64, N], F32, tag="tagA")
            matmul_tile_kernel(tc, wtaga[:, :], hout2[:, :], tagA[:, :])
            tagB = dram.tile([64, N], F32, tag="tagB")
            matmul_tile_kernel(tc, wtagb[:, :], ag2[:, :], tagB[:, :])
            CH = min(2048, N)
            for j in range(N // CH):
                ta = cons.tile([64, CH], F32, tag="ta")
                tb = cons.tile([64, CH], F32, tag="tb")
                nc.sync.dma_start(out=ta[:, :], in_=tagA[:, ts(j, CH)])
                nbk = CH // 32
                sbv = tagB[:, :].rearrange("r (s b) -> r s b", b=32)
                src = sbv[:, ts(N // CH - 1 - j, nbk), :]
                rsrc = bass.AP(src.tensor, src.offset + (nbk - 1) * 32,
                               [[N, 64], [-32, nbk], [1, 32]])
                nc.sync.dma_start(out=tb[:, :], in_=rsrc)
                nc.vector.tensor_add(out=ta[:, :], in0=ta[:, :], in1=tb[:, :])
                nc.sync.dma_start(out=tags[:, ts(j, CH)], in_=ta[:, :])
    nc.compile()
    return nc


def _bf(x):
    return np.ascontiguousarray(x).astype(ml_dtypes.bfloat16)


def _pack_inputs(words, emb, Wf1, bf1, Wb1, bb1, Wf2, bf2, Wb2, bb2, Wtag):
    words = np.asarray(words).astype(np.int64)
    xs = np.asarray(emb, dtype=np.float32)[words]      # [B, S, E] host gather
    xsT_f = xs.transpose(2, 1, 0).reshape(512, S * B)  # col = t*B + b
    xsT_b = xs[:, ::-1].transpose(2, 1, 0).reshape(512, S * B)

    def pack_xs(xsT):
        out = np.zeros((640, xsT.shape[1]), np.float32)
        out[:512] = xsT
        out[512] = 1.0
        return _bf(out)

    def gates_cat(W, rows):
        return np.concatenate([np.asarray(W[g], dtype=np.float32)[rows]
                               for g in GATE_PERM], axis=1)

    def bias_cat(b):
        return np.concatenate([np.asarray(b[g], dtype=np.float32) for g in GATE_PERM])

    def pack_dir(Wl1, bl1, Wl2, bl2, fwd):
        wx1p = np.zeros((640, 2048), np.float32)
        wx1p[:512] = gates_cat(Wl1, slice(0, 512))
        wx1p[512] = bias_cat(bl1)
        wh1p = gates_cat(Wl1, slice(512, 1024))
        ownsl = slice(0, 512) if fwd else slice(512, 1024)
        wx2m = gates_cat(Wl2, slice(0, 1024))
        wx2ap = np.zeros((640, 2048), np.float32)
        wx2ap[:512] = wx2m[ownsl]
        wx2ap[512] = bias_cat(bl2)
        wx2bp = wx2m.copy()
        wx2bp[ownsl] = 0.0
        wh2p = gates_cat(Wl2, slice(1024, 1536))
        wt = np.asarray(Wtag, dtype=np.float32)
        wtagap = wt[ownsl]
        wtagbp = wt.copy()
        wtagbp[ownsl] = 0.0
        return dict(
            xsT=pack_xs(xsT_f if fwd else xsT_b),
            wx1=_bf(wx1p), wh1=_bf(wh1p),
            wx2a=_bf(wx2ap), wx2b=_bf(wx2bp), wh2=_bf(wh2p),
            wtaga=_bf(wtagap), wtagb=_bf(wtagbp),
        )

    fw = pack_dir(Wf1, bf1, Wf2, bf2, True)
    bw = pack_dir(Wb1, bb1, Wb2, bb2, False)
    return [fw] * 4 + [bw] * 4


_NC_CACHE = {}


def _get_nc():
    if "nc" not in _NC_CACHE:
        _NC_CACHE["nc"] = _build_full()
    return _NC_CACHE["nc"]


def kernel(words, lengths, emb, Wf1, bf1, Wb1, bb1, Wf2, bf2, Wb2, bb2, Wtag, btag):
    nc = _get_nc()
    ins = _pack_inputs(words, emb, Wf1, bf1, Wb1, bb1, Wf2, bf2, Wb2, bb2, Wtag)
    res = run_bass_kernel_spmd(nc, ins, core_ids=list(range(8)))
    tags = res.results[0]["tags"]                       # [64, S*B], col = t*B+b
    out = tags.reshape(T, S, B).transpose(2, 1, 0).reshape(B * S, T)
    out = out + np.asarray(btag, dtype=np.float32)[None, :]
    return np.ascontiguousarray(out.astype(np.float32))
